# revision 4
# baseline (speedup 1.0000x reference)
"""Trainium2 Bass kernel for nn_MultiHeadModulator (8-core SPMD).

Math reformulation (exact): with a single query q = Wq@z_curr+bq,
  - dot scores:  score[l,h] = z[l]·A[:,h] + c[h],   A[:,h] = Wk[hb,:]^T @ q[hb]
  - rel scores fold into a per-(l,h) additive bias known on the host
  - value sum:   sum_l e[l,h]*v[l] = Wv @ (sum_l e[l,h]*z[l]) + (sum_l e[l,h])*bv
so the device only computes, per L-shard:
  score^T = A^T z^T   (PE, fp8 DoubleRow),  e^T = exp(scale*score + c_h) * fac
  U[h,:] += e^T z     (PE, fp8 DoubleRow),  S[h] from exp's accum_out
and the host applies Wv/Wo and the softmax normalization to the tiny [8,512]
all-core sums.  Softmax runs without max-subtraction: scores are O(1) by
construction (validated |score| < 3).

Sharding: z_past split into 8 contiguous shards of 8192 rows, one per core.
The host ships each shard twice (feature-major for scores, row-major for U)
in fp8, pre-packed for DoubleRow access patterns (the dual layout costs 2x
HBM but avoids any on-chip transpose of z; only the tiny e^T [8,512] tiles
get PE-transposed per block).

Scheduling notes (hard-won):
  - each dma_start costs ~0.65us of SERIAL sequencer dispatch (DIRECT2D).
    With all 36 triggers on the sync ring the dispatch alone paces the
    kernel (~23us).  v2 splits triggers across the two HWDGE rings:
    scalar (boots ~2.5us) carries block-granular early loads so the first
    score matmul can start ~6us; sync (boots ~7us) carries supers-of-2
    for blocks 4-15.  All bulk triggers are dispatched before the compute
    loop (bulk DMAs emitted inside the loop get interleaved AFTER exp
    instructions on the scalar sequencer and stall).
  - gpsimd/SWDGE first-byte is ~10 us - never put early loads there.
  - weight-side DoubleRow LDWEIGHTS requires the pair-dim step to be a
    multiple of 16 elements (hence the [.., 16]-padded e8 layout).
  - nc.vector.tensor_tensor_reduce crashes on HW (fine in CoreSim); S uses
    the exp's accum_out for uncorrected blocks + a DVE reduce for block 0.
  - a_dr rides in the zt0 tile (cols 512:528) - one fewer trigger; cb is
    shipped bf16 inside the const tile (a per-head-constant bias error
    cancels in the U/S ratio).
  - PSUM budget (8 banks): 4x score + 3x e-transpose + 1x U accumulator.
"""

import numpy as np
import ml_dtypes

import concourse.bass as bass  # noqa: F401  (engine namespaces live on the nc)
import concourse.mybir as mybir
import concourse.tile as tile
from concourse import bacc
from concourse.bass_utils import run_bass_kernel_spmd

HEADS = 8
REL_MAX = 64
DIM = 256
D2 = 512                      # flattened real feature dim
HD = DIM // HEADS             # 32 complex => 64 reals per head block
L_TOTAL = 65536
N_CORES = 8
L_SHARD = L_TOTAL // N_CORES  # 8192
N_BLOCKS = L_SHARD // 512     # 16 blocks of 512 rows
N_SING = 4                    # leading blocks shipped as single-block DMAs
BLK_PER_SUPER = 2             # blocks per bulk DMA for the rest
N_SUPER = (N_BLOCKS - N_SING) // BLK_PER_SUPER
SCALE = 1.0 / np.sqrt(HD)

FP8 = ml_dtypes.float8_e4m3   # == mybir.dt.float8e4 (trainium E4M3, max 240)
BF16 = ml_dtypes.bfloat16

TRACE = False                 # test.py can flip this for profiling runs
TRACE_KW = {}

_cached = {}


def _build_program(full_fac: bool):
    nc = bacc.Bacc(
        "TRN2", target_bir_lowering=False, debug=False, num_devices=N_CORES
    )
    DR = mybir.MatmulPerfMode.DoubleRow
    f8 = mybir.dt.float8e4
    facw = L_SHARD if full_fac else 512

    # block 0 of zt with a_dr packed into cols 512:528 of the last axis
    ZT0 = nc.dram_tensor("zt0", [128, 2, 2, 528], f8, kind="ExternalInput")
    ZT1 = nc.dram_tensor(
        "zt1", [N_SING - 1, 128, 2, 2, 512], f8, kind="ExternalInput"
    )
    ZTS = nc.dram_tensor(
        "zts", [N_SUPER, 128, BLK_PER_SUPER, 2, 2, 512], f8, kind="ExternalInput"
    )
    ZN1 = nc.dram_tensor(
        "zn1", [N_SING, 128, 2, 2, 512], f8, kind="ExternalInput"
    )
    ZNS = nc.dram_tensor(
        "zns", [N_SUPER, 128, BLK_PER_SUPER, 2, 2, 512], f8, kind="ExternalInput"
    )
    # col 0: cb (bf16; per-head-constant error cancels in U/S), 1:9 identity,
    # 9:9+facw rel-bias correction factors
    CST = nc.dram_tensor("cst", [8, 9 + facw], mybir.dt.bfloat16,
                         kind="ExternalInput")
    OUT_U = nc.dram_tensor("out_u", [8, 512], mybir.dt.float32,
                           kind="ExternalOutput")
    OUT_S = nc.dram_tensor("out_s", [8, N_BLOCKS], mybir.dt.float32,
                           kind="ExternalOutput")

    with tile.TileContext(nc) as tc:
        with (
            tc.tile_pool(name="zt0", bufs=1) as zt0_pool,
            tc.tile_pool(name="zt1", bufs=N_SING - 1) as zt1_pool,
            tc.tile_pool(name="zts", bufs=N_SUPER) as zts_pool,
            tc.tile_pool(name="zn1", bufs=N_SING) as zn1_pool,
            tc.tile_pool(name="zns", bufs=N_SUPER) as zns_pool,
            tc.tile_pool(name="consts", bufs=1) as const_pool,
            tc.tile_pool(name="et", bufs=6) as et_pool,
            tc.tile_pool(name="e8", bufs=6) as e8_pool,
            tc.tile_pool(name="outs", bufs=1) as out_pool,
            tc.tile_pool(name="ps_sc", bufs=4, space="PSUM") as sc_pool,
            tc.tile_pool(name="ps_etp", bufs=3, space="PSUM") as etp_pool,
            tc.tile_pool(name="ps_acc", bufs=1, space="PSUM") as acc_pool,
        ):
            # ---- scalar-ring triggers (boots ~2.5us): early blocks ----
            zt0_sb = zt0_pool.tile([128, 2, 2, 528], f8)
            nc.scalar.dma_start(zt0_sb[:], ZT0[:])
            zn1_tiles = [
                zn1_pool.tile([128, 2, 2, 512], f8, tag="zn1", name=f"zn1_{i}")
                for i in range(N_SING)
            ]
            zt1_tiles = [
                zt1_pool.tile([128, 2, 2, 512], f8, tag="zt1", name=f"zt1_{i}")
                for i in range(N_SING - 1)
            ]
            cst_sb = const_pool.tile([8, 9 + facw], mybir.dt.bfloat16)
            nc.scalar.dma_start(zn1_tiles[0][:], ZN1[0])
            nc.scalar.dma_start(cst_sb[:], CST[:])
            nc.scalar.dma_start(zt1_tiles[0][:], ZT1[0])
            nc.scalar.dma_start(zn1_tiles[1][:], ZN1[1])
            nc.scalar.dma_start(zt1_tiles[1][:], ZT1[1])
            nc.scalar.dma_start(zn1_tiles[2][:], ZN1[2])
            nc.scalar.dma_start(zt1_tiles[2][:], ZT1[2])
            nc.scalar.dma_start(zn1_tiles[3][:], ZN1[3])

            # ---- sync-ring triggers (boots ~7us): supers of 2 blocks ----
            zts_tiles = [None] * N_SUPER
            zns_tiles = [None] * N_SUPER
            for s in range(N_SUPER):
                zts_tiles[s] = zts_pool.tile(
                    [128, BLK_PER_SUPER, 2, 2, 512], f8, tag="zts",
                    name=f"zts_{s}",
                )
                nc.sync.dma_start(zts_tiles[s][:], ZTS[s])
                zns_tiles[s] = zns_pool.tile(
                    [128, BLK_PER_SUPER, 2, 2, 512], f8, tag="zns",
                    name=f"zns_{s}",
                )
                nc.sync.dma_start(zns_tiles[s][:], ZNS[s])

            u_ps = acc_pool.tile([8, 512], mybir.dt.float32)
            outs_sb = out_pool.tile([8, N_BLOCKS], mybir.dt.float32)
            u_sb = out_pool.tile([8, 512], mybir.dt.float32)

            for b in range(N_BLOCKS):
                if b == 0:
                    zt_t = zt0_sb[:, :, :, 0:512]
                elif b < N_SING:
                    zt_t = zt1_tiles[b - 1]
                else:
                    s, blk = divmod(b - N_SING, BLK_PER_SUPER)
                    zt_t = zts_tiles[s][:, blk]
                if b < N_SING:
                    zn_t = zn1_tiles[b]
                else:
                    s, blk = divmod(b - N_SING, BLK_PER_SUPER)
                    zn_t = zns_tiles[s][:, blk]

                # score^T[h, l] for this block's 512 rows, K=512 via 2x DoubleRow
                sc = sc_pool.tile([8, 512], mybir.dt.float32)
                for cpair in range(2):
                    nc.tensor.matmul(
                        sc[:],
                        zt0_sb[:, cpair, :, 512:520],
                        zt_t[:, cpair] if b else zt0_sb[:, cpair, :, 0:512],
                        start=(cpair == 0),
                        stop=(cpair == 1),
                        perf_mode=DR,
                    )

                et = et_pool.tile([8, 512], mybir.dt.bfloat16, tag="et")
                # for fac==1 blocks, S comes free from the exp's accum_out
                accum = (
                    {}
                    if (full_fac or b == 0)
                    else {"accum_out": outs_sb[:, b : b + 1]}
                )
                nc.scalar.activation(
                    et[:],
                    sc[:],
                    mybir.ActivationFunctionType.Exp,
                    bias=cst_sb[:, 0:1],
                    scale=float(SCALE),
                    **accum,
                )
                # rel-bias correction factors: only block 0 deviates from 1
                # in the common curr_pos regime (full_fac covers the rest)
                if full_fac or b == 0:
                    etc = et_pool.tile([8, 512], mybir.dt.bfloat16, tag="etc")
                    nc.vector.tensor_mul(
                        etc[:], et[:], cst_sb[:, 9 + 512 * b : 9 + 512 * (b + 1)]
                    )
                    # S for corrected blocks: one DVE free-axis reduction
                    nc.vector.tensor_reduce(
                        outs_sb[:, b : b + 1],
                        etc[:],
                        axis=mybir.AxisListType.X,
                        op=mybir.AluOpType.add,
                    )
                else:
                    etc = et

                # transpose e^T -> e[l,h] in 4x [8,128] chunks (PE via identity)
                etp = etp_pool.tile([128, 4, 8], mybir.dt.bfloat16)
                for quad in range(4):
                    nc.tensor.transpose(
                        etp[:, quad],
                        etc[:, 128 * quad : 128 * (quad + 1)],
                        cst_sb[:, 1:9],
                    )
                e8 = e8_pool.tile([128, 4, 16], f8)
                nc.vector.tensor_copy(e8[:, :, 0:8], etp[:])

                for s in range(2):
                    nc.tensor.matmul(
                        u_ps[:],
                        e8[:, 2 * s : 2 * s + 2, 0:8],
                        zn_t[:, s],
                        start=(b == 0 and s == 0),
                        stop=(b == N_BLOCKS - 1 and s == 1),
                        perf_mode=DR,
                    )

            # S partials ride the idle sync ring; the ACT engine (closest to
            # PSUM, free after the last exp) copies U and fires its DMA
            nc.sync.dma_start(OUT_S[:], outs_sb[:])
            nc.scalar.copy(u_sb[:], u_ps[:])
            nc.scalar.dma_start(OUT_U[:], u_sb[:])

    nc.compile()
    return nc


def _get_program(full_fac: bool):
    if full_fac not in _cached:
        _cached[full_fac] = _build_program(full_fac)
    return _cached[full_fac]


def kernel(curr_pos, z_curr, z_past, Wq, bq, Wk, bk, Wv, bv, Wo, bo, rel_bias):
    curr_pos = int(np.asarray(curr_pos))
    z_curr = np.asarray(z_curr, dtype=np.float32)
    z_past = np.asarray(z_past, dtype=np.float32)
    Wq = np.asarray(Wq, dtype=np.float32)
    bq = np.asarray(bq, dtype=np.float32)
    Wk = np.asarray(Wk, dtype=np.float32)
    bk = np.asarray(bk, dtype=np.float32)
    Wv = np.asarray(Wv, dtype=np.float32)
    bv = np.asarray(bv, dtype=np.float32)
    Wo = np.asarray(Wo, dtype=np.float32)
    bo = np.asarray(bo, dtype=np.float32)
    rel_bias = np.asarray(rel_bias, dtype=np.float32)

    # ---- host-side O(D^2) prep (f64) ----
    q = z_curr.reshape(-1).astype(np.float64) @ Wq.T.astype(np.float64) + bq
    A = np.zeros((D2, HEADS), np.float64)
    c = np.zeros(HEADS, np.float64)
    for h in range(HEADS):
        sl = slice(h * 2 * HD, (h + 1) * 2 * HD)
        A[:, h] = Wk[sl, :].T.astype(np.float64) @ q[sl]
        c[h] = bk[sl].astype(np.float64) @ q[sl]
    relflat = rel_bias.reshape(2 * REL_MAX + 1, D2).astype(np.float64)
    rb = np.stack(
        [
            relflat[:, h * 2 * HD : (h + 1) * 2 * HD] @ q[h * 2 * HD : (h + 1) * 2 * HD]
            for h in range(HEADS)
        ],
        axis=1,
    )  # [129, 8]
    idx = np.clip(
        curr_pos - L_TOTAL + np.arange(L_TOTAL) + REL_MAX, 0, 2 * REL_MAX
    ).astype(np.int64)

    z8 = np.clip(z_past.reshape(L_TOTAL, D2), -240.0, 240.0).astype(FP8)
    A8 = np.clip(A, -240.0, 240.0).astype(np.float32).astype(FP8)
    a_dr = np.zeros((128, 2, 2, 16), FP8)
    a_dr[:, :, :, 0:8] = A8.reshape(2, 2, 128, HEADS).transpose(2, 0, 1, 3)

    in_maps = []
    facs = []
    for core in range(N_CORES):
        zc = z8[core * L_SHARD : (core + 1) * L_SHARD]
        # zt_b[p, cpair, d, l] = zc[512*b + l, 256*cpair + 128*d + p]
        zt_all = np.ascontiguousarray(
            zc.reshape(N_BLOCKS, 512, 2, 2, 128).transpose(0, 4, 2, 3, 1)
        )
        # zn_b[p, s, d, f] = zc[512*b + 256*s + 128*d + p, f]
        zn_all = np.ascontiguousarray(
            zc.reshape(N_BLOCKS, 2, 2, 128, 512).transpose(0, 3, 1, 2, 4)
        )
        zt0 = np.concatenate([zt_all[0], a_dr], axis=3)
        zts = np.ascontiguousarray(
            zt_all[N_SING:]
            .reshape(N_SUPER, BLK_PER_SUPER, 128, 2, 2, 512)
            .transpose(0, 2, 1, 3, 4, 5)
        )
        zns = np.ascontiguousarray(
            zn_all[N_SING:]
            .reshape(N_SUPER, BLK_PER_SUPER, 128, 2, 2, 512)
            .transpose(0, 2, 1, 3, 4, 5)
        )
        idx_c = idx[core * L_SHARD : (core + 1) * L_SHARD]
        base = int(np.bincount(idx_c, minlength=2 * REL_MAX + 1).argmax())
        cb = ((c + rb[base]) * SCALE).astype(np.float32).reshape(HEADS, 1)
        fac = np.ascontiguousarray(
            np.exp((rb[idx_c] - rb[base]) * SCALE).T.astype(BF16)
        )
        facs.append(fac)
        in_maps.append(
            {
                "zt0": zt0,
                "zt1": np.ascontiguousarray(zt_all[1:N_SING]),
                "zts": zts,
                "zn1": np.ascontiguousarray(zn_all[0:N_SING]),
                "zns": zns,
                "cb": cb,
            }
        )

    # fast path: correction factors are 1.0 outside block 0 on every core
    full_fac = any(
        not np.all(f[:, 512:] == np.asarray(1.0, BF16)) for f in facs
    )
    facw = L_SHARD if full_fac else 512
    for core, m in enumerate(in_maps):
        cst = np.zeros((8, 9 + facw), BF16)
        cst[:, 0:1] = m.pop("cb").astype(BF16)
        cst[:, 1:9] = np.eye(8, dtype=BF16)
        cst[:, 9:] = facs[core][:, 0:facw]
        m["cst"] = cst

    nc = _get_program(full_fac)
    res = run_bass_kernel_spmd(
        nc, in_maps, list(range(N_CORES)), trace=TRACE, **TRACE_KW
    )
    if TRACE:
        kernel.last_result = res

    U = np.zeros((HEADS, D2), np.float64)
    S = np.zeros(HEADS, np.float64)
    for r in res.results:
        U += np.asarray(r["out_u"], dtype=np.float64)
        S += np.asarray(r["out_s"], dtype=np.float64).sum(axis=1)

    hvec = np.zeros(D2, np.float64)
    for h in range(HEADS):
        sl = slice(h * 2 * HD, (h + 1) * 2 * HD)
        hvec[sl] = Wv[sl, :].astype(np.float64) @ (U[h] / S[h]) + bv[sl]
    out = hvec @ Wo.T.astype(np.float64) + bo
    return out.reshape(DIM, 2).astype(np.float32)


# revision 10
# speedup vs baseline: 1.0797x; 1.0797x over previous
"""Trainium2 Bass kernel for nn_MultiHeadModulator (8-core SPMD).

Math reformulation (exact): with a single query q = Wq@z_curr+bq,
  - dot scores:  score[l,h] = z[l]·A[:,h] + c[h],   A[:,h] = Wk[hb,:]^T @ q[hb]
  - rel scores fold into a per-(l,h) additive bias known on the host
  - value sum:   sum_l e[l,h]*v[l] = Wv @ (sum_l e[l,h]*z[l]) + (sum_l e[l,h])*bv
so the device only computes, per L-shard:
  score^T = A^T z^T   (PE, fp8 DoubleRow),  e^T = exp(scale*score + c_h) * fac
  U[h,:] += e^T z     (PE, fp8 DoubleRow),  S[h] from exp's accum_out
and the host applies Wv/Wo and the softmax normalization to the tiny [8,512]
all-core sums.  Softmax runs without max-subtraction: scores are O(1) by
construction (validated |score| < 3).

Sharding: z_past split into 8 contiguous shards of 8192 rows, one per core.
The host ships each shard twice (feature-major for scores, row-major for U)
in fp8, pre-packed for DoubleRow access patterns (the dual layout costs 2x
HBM but avoids any on-chip transpose of z; only the tiny e^T [8,512] tiles
get PE-transposed per block).

Scheduling notes (hard-won):
  - each dma_start costs ~0.65us of SERIAL sequencer dispatch (DIRECT2D).
    With all 36 triggers on the sync ring the dispatch alone paces the
    kernel (~23us).  v2 splits triggers across the two HWDGE rings:
    scalar (boots ~2.5us) carries block-granular early loads so the first
    score matmul can start ~6us; sync (boots ~7us) carries supers-of-2
    for blocks 4-15.  All bulk triggers are dispatched before the compute
    loop (bulk DMAs emitted inside the loop get interleaved AFTER exp
    instructions on the scalar sequencer and stall).
  - gpsimd/SWDGE first-byte is ~10 us - never put early loads there.
  - weight-side DoubleRow LDWEIGHTS requires the pair-dim step to be a
    multiple of 16 elements (hence the [.., 16]-padded e8 layout).
  - nc.vector.tensor_tensor_reduce crashes on HW (fine in CoreSim); S uses
    the exp's accum_out for uncorrected blocks + a DVE reduce for block 0.
  - a_dr rides in the zt0 tile (cols 512:528) - one fewer trigger; cb is
    shipped bf16 inside the const tile (a per-head-constant bias error
    cancels in the U/S ratio).
  - PSUM budget (8 banks): 4x score + 3x e-transpose + 1x U accumulator.
"""

import numpy as np
import ml_dtypes

import concourse.bass as bass  # noqa: F401  (engine namespaces live on the nc)
import concourse.mybir as mybir
import concourse.tile as tile
from concourse import bacc
from concourse.bass_utils import run_bass_kernel_spmd

HEADS = 8
REL_MAX = 64
DIM = 256
D2 = 512                      # flattened real feature dim
HD = DIM // HEADS             # 32 complex => 64 reals per head block
L_TOTAL = 65536
N_CORES = 8
L_SHARD = L_TOTAL // N_CORES  # 8192
N_BLOCKS = L_SHARD // 512     # 16 blocks of 512 rows
N_SING = 4                    # leading blocks shipped in fine-grained DMAs
BLK_PER_SUPER = 4             # blocks per bulk DMA for the rest
N_SUPER = (N_BLOCKS - N_SING) // BLK_PER_SUPER
SCALE = 1.0 / np.sqrt(HD)

FP8 = ml_dtypes.float8_e4m3   # == mybir.dt.float8e4 (trainium E4M3, max 240)
BF16 = ml_dtypes.bfloat16

TRACE = False                 # test.py can flip this for profiling runs
TRACE_KW = {}

_cached = {}


def _build_program(full_fac: bool):
    nc = bacc.Bacc(
        "TRN2", target_bir_lowering=False, debug=False, num_devices=N_CORES
    )
    DR = mybir.MatmulPerfMode.DoubleRow
    f8 = mybir.dt.float8e4
    facw = L_SHARD if full_fac else 512

    # block 0 of zt with a_dr packed into cols 512:528 of the last axis
    ZT0 = nc.dram_tensor("zt0", [128, 2, 2, 528], f8, kind="ExternalInput")
    ZT1 = nc.dram_tensor(
        "zt1", [128, N_SING - 1, 2, 2, 512], f8, kind="ExternalInput"
    )
    ZTS = nc.dram_tensor(
        "zts", [N_SUPER, 128, BLK_PER_SUPER, 2, 2, 512], f8, kind="ExternalInput"
    )
    ZN1 = nc.dram_tensor(
        "zn1", [128, N_SING, 2, 2, 512], f8, kind="ExternalInput"
    )
    ZNS = nc.dram_tensor(
        "zns", [N_SUPER, 128, BLK_PER_SUPER, 2, 2, 512], f8, kind="ExternalInput"
    )
    # col 0: cb (bf16; per-head-constant error cancels in U/S), 1:9 identity,
    # 9:9+facw rel-bias correction factors
    CST = nc.dram_tensor("cst", [8, 9 + facw], mybir.dt.bfloat16,
                         kind="ExternalInput")
    OUT_U = nc.dram_tensor("out_u", [8, 512], mybir.dt.float32,
                           kind="ExternalOutput")
    OUT_S = nc.dram_tensor("out_s", [8, N_BLOCKS], mybir.dt.float32,
                           kind="ExternalOutput")

    with tile.TileContext(nc) as tc:
        with (
            tc.tile_pool(name="zt0", bufs=1) as zt0_pool,
            tc.tile_pool(name="zt1", bufs=1) as zt1_pool,
            tc.tile_pool(name="zts", bufs=N_SUPER) as zts_pool,
            tc.tile_pool(name="zn1", bufs=1) as zn1_pool,
            tc.tile_pool(name="zns", bufs=N_SUPER) as zns_pool,
            tc.tile_pool(name="consts", bufs=1) as const_pool,
            tc.tile_pool(name="et", bufs=6) as et_pool,
            tc.tile_pool(name="e8", bufs=6) as e8_pool,
            tc.tile_pool(name="outs", bufs=1) as out_pool,
            tc.tile_pool(name="ps_sc", bufs=4, space="PSUM") as sc_pool,
            tc.tile_pool(name="ps_etp", bufs=3, space="PSUM") as etp_pool,
            tc.tile_pool(name="ps_acc", bufs=1, space="PSUM") as acc_pool,
        ):
            # ---- all bulk rides the sync HWDGE ring (the only fast one);
            # ~10 large triggers instead of 35 small ones: each DIRECT2D
            # costs ~0.65us of serial dispatch + ring-credit coupling, and
            # that dispatch rate is what paced the 41.5us baseline ----
            zt0_sb = zt0_pool.tile([128, 2, 2, 528], f8)
            nc.sync.dma_start(zt0_sb[:], ZT0[:])
            cst_sb = const_pool.tile([8, 9 + facw], mybir.dt.bfloat16)
            nc.sync.dma_start(cst_sb[:], CST[:])
            zt1_sb = zt1_pool.tile([128, N_SING - 1, 2, 2, 512], f8)
            nc.sync.dma_start(zt1_sb[:], ZT1[:])
            zn1_sb = zn1_pool.tile([128, N_SING, 2, 2, 512], f8)
            nc.sync.dma_start(zn1_sb[:], ZN1[:])
            zts_tiles = [None] * N_SUPER
            zns_tiles = [None] * N_SUPER
            for s in range(N_SUPER):
                zts_tiles[s] = zts_pool.tile(
                    [128, BLK_PER_SUPER, 2, 2, 512], f8, tag="zts",
                    name=f"zts_{s}",
                )
                nc.sync.dma_start(zts_tiles[s][:], ZTS[s])
                zns_tiles[s] = zns_pool.tile(
                    [128, BLK_PER_SUPER, 2, 2, 512], f8, tag="zns",
                    name=f"zns_{s}",
                )
                nc.sync.dma_start(zns_tiles[s][:], ZNS[s])

            u_ps = acc_pool.tile([8, 512], mybir.dt.float32)
            outs_sb = out_pool.tile([8, N_BLOCKS], mybir.dt.float32)
            u_sb = out_pool.tile([8, 512], mybir.dt.float32)

            for b in range(N_BLOCKS):
                if b == 0:
                    zt_t = zt0_sb[:, :, :, 0:512]
                elif b < N_SING:
                    zt_t = zt1_sb[:, b - 1]
                else:
                    s, blk = divmod(b - N_SING, BLK_PER_SUPER)
                    zt_t = zts_tiles[s][:, blk]
                if b < N_SING:
                    zn_t = zn1_sb[:, b]
                else:
                    s, blk = divmod(b - N_SING, BLK_PER_SUPER)
                    zn_t = zns_tiles[s][:, blk]

                # score^T[h, l] for this block's 512 rows, K=512 via 2x DoubleRow
                sc = sc_pool.tile([8, 512], mybir.dt.float32)
                for cpair in range(2):
                    nc.tensor.matmul(
                        sc[:],
                        zt0_sb[:, cpair, :, 512:520],
                        zt_t[:, cpair] if b else zt0_sb[:, cpair, :, 0:512],
                        start=(cpair == 0),
                        stop=(cpair == 1),
                        perf_mode=DR,
                    )

                et = et_pool.tile([8, 512], mybir.dt.bfloat16, tag="et")
                # for fac==1 blocks, S comes free from the exp's accum_out
                accum = (
                    {}
                    if (full_fac or b == 0)
                    else {"accum_out": outs_sb[:, b : b + 1]}
                )
                nc.scalar.activation(
                    et[:],
                    sc[:],
                    mybir.ActivationFunctionType.Exp,
                    bias=cst_sb[:, 0:1],
                    scale=float(SCALE),
                    **accum,
                )
                # rel-bias correction factors: only block 0 deviates from 1
                # in the common curr_pos regime (full_fac covers the rest)
                if full_fac or b == 0:
                    etc = et_pool.tile([8, 512], mybir.dt.bfloat16, tag="etc")
                    nc.vector.tensor_mul(
                        etc[:], et[:], cst_sb[:, 9 + 512 * b : 9 + 512 * (b + 1)]
                    )
                    # S for corrected blocks: one DVE free-axis reduction
                    nc.vector.tensor_reduce(
                        outs_sb[:, b : b + 1],
                        etc[:],
                        axis=mybir.AxisListType.X,
                        op=mybir.AluOpType.add,
                    )
                else:
                    etc = et

                # transpose e^T -> e[l,h] in 4x [8,128] chunks (PE via identity)
                etp = etp_pool.tile([128, 4, 8], mybir.dt.bfloat16)
                for quad in range(4):
                    nc.tensor.transpose(
                        etp[:, quad],
                        etc[:, 128 * quad : 128 * (quad + 1)],
                        cst_sb[:, 1:9],
                    )
                e8 = e8_pool.tile([128, 4, 16], f8)
                nc.vector.tensor_copy(e8[:, :, 0:8], etp[:])

                for s in range(2):
                    nc.tensor.matmul(
                        u_ps[:],
                        e8[:, 2 * s : 2 * s + 2, 0:8],
                        zn_t[:, s],
                        start=(b == 0 and s == 0),
                        stop=(b == N_BLOCKS - 1 and s == 1),
                        perf_mode=DR,
                    )

            # S partials ride the idle sync ring; the ACT engine (closest to
            # PSUM, free after the last exp) copies U and fires its DMA
            nc.sync.dma_start(OUT_S[:], outs_sb[:])
            nc.scalar.copy(u_sb[:], u_ps[:])
            nc.scalar.dma_start(OUT_U[:], u_sb[:])

    nc.compile()
    return nc


def _get_program(full_fac: bool):
    if full_fac not in _cached:
        _cached[full_fac] = _build_program(full_fac)
    return _cached[full_fac]


def kernel(curr_pos, z_curr, z_past, Wq, bq, Wk, bk, Wv, bv, Wo, bo, rel_bias):
    curr_pos = int(np.asarray(curr_pos))
    z_curr = np.asarray(z_curr, dtype=np.float32)
    z_past = np.asarray(z_past, dtype=np.float32)
    Wq = np.asarray(Wq, dtype=np.float32)
    bq = np.asarray(bq, dtype=np.float32)
    Wk = np.asarray(Wk, dtype=np.float32)
    bk = np.asarray(bk, dtype=np.float32)
    Wv = np.asarray(Wv, dtype=np.float32)
    bv = np.asarray(bv, dtype=np.float32)
    Wo = np.asarray(Wo, dtype=np.float32)
    bo = np.asarray(bo, dtype=np.float32)
    rel_bias = np.asarray(rel_bias, dtype=np.float32)

    # ---- host-side O(D^2) prep (f64) ----
    q = z_curr.reshape(-1).astype(np.float64) @ Wq.T.astype(np.float64) + bq
    A = np.zeros((D2, HEADS), np.float64)
    c = np.zeros(HEADS, np.float64)
    for h in range(HEADS):
        sl = slice(h * 2 * HD, (h + 1) * 2 * HD)
        A[:, h] = Wk[sl, :].T.astype(np.float64) @ q[sl]
        c[h] = bk[sl].astype(np.float64) @ q[sl]
    relflat = rel_bias.reshape(2 * REL_MAX + 1, D2).astype(np.float64)
    rb = np.stack(
        [
            relflat[:, h * 2 * HD : (h + 1) * 2 * HD] @ q[h * 2 * HD : (h + 1) * 2 * HD]
            for h in range(HEADS)
        ],
        axis=1,
    )  # [129, 8]
    idx = np.clip(
        curr_pos - L_TOTAL + np.arange(L_TOTAL) + REL_MAX, 0, 2 * REL_MAX
    ).astype(np.int64)

    z8 = np.clip(z_past.reshape(L_TOTAL, D2), -240.0, 240.0).astype(FP8)
    A8 = np.clip(A, -240.0, 240.0).astype(np.float32).astype(FP8)
    a_dr = np.zeros((128, 2, 2, 16), FP8)
    a_dr[:, :, :, 0:8] = A8.reshape(2, 2, 128, HEADS).transpose(2, 0, 1, 3)

    in_maps = []
    facs = []
    for core in range(N_CORES):
        zc = z8[core * L_SHARD : (core + 1) * L_SHARD]
        # zt_b[p, cpair, d, l] = zc[512*b + l, 256*cpair + 128*d + p]
        zt_all = np.ascontiguousarray(
            zc.reshape(N_BLOCKS, 512, 2, 2, 128).transpose(0, 4, 2, 3, 1)
        )
        # zn_b[p, s, d, f] = zc[512*b + 256*s + 128*d + p, f]
        zn_all = np.ascontiguousarray(
            zc.reshape(N_BLOCKS, 2, 2, 128, 512).transpose(0, 3, 1, 2, 4)
        )
        zt0 = np.concatenate([zt_all[0], a_dr], axis=3)
        zts = np.ascontiguousarray(
            zt_all[N_SING:]
            .reshape(N_SUPER, BLK_PER_SUPER, 128, 2, 2, 512)
            .transpose(0, 2, 1, 3, 4, 5)
        )
        zns = np.ascontiguousarray(
            zn_all[N_SING:]
            .reshape(N_SUPER, BLK_PER_SUPER, 128, 2, 2, 512)
            .transpose(0, 2, 1, 3, 4, 5)
        )
        idx_c = idx[core * L_SHARD : (core + 1) * L_SHARD]
        base = int(np.bincount(idx_c, minlength=2 * REL_MAX + 1).argmax())
        cb = ((c + rb[base]) * SCALE).astype(np.float32).reshape(HEADS, 1)
        fac = np.ascontiguousarray(
            np.exp((rb[idx_c] - rb[base]) * SCALE).T.astype(BF16)
        )
        facs.append(fac)
        in_maps.append(
            {
                "zt0": zt0,
                "zt1": np.ascontiguousarray(
                    zt_all[1:N_SING].transpose(1, 0, 2, 3, 4)
                ),
                "zts": zts,
                "zn1": np.ascontiguousarray(
                    zn_all[0:N_SING].transpose(1, 0, 2, 3, 4)
                ),
                "zns": zns,
                "cb": cb,
            }
        )

    # fast path: correction factors are 1.0 outside block 0 on every core
    full_fac = any(
        not np.all(f[:, 512:] == np.asarray(1.0, BF16)) for f in facs
    )
    facw = L_SHARD if full_fac else 512
    for core, m in enumerate(in_maps):
        cst = np.zeros((8, 9 + facw), BF16)
        cst[:, 0:1] = m.pop("cb").astype(BF16)
        cst[:, 1:9] = np.eye(8, dtype=BF16)
        cst[:, 9:] = facs[core][:, 0:facw]
        m["cst"] = cst

    nc = _get_program(full_fac)
    res = run_bass_kernel_spmd(
        nc, in_maps, list(range(N_CORES)), trace=TRACE, **TRACE_KW
    )
    if TRACE:
        kernel.last_result = res

    U = np.zeros((HEADS, D2), np.float64)
    S = np.zeros(HEADS, np.float64)
    for r in res.results:
        U += np.asarray(r["out_u"], dtype=np.float64)
        S += np.asarray(r["out_s"], dtype=np.float64).sum(axis=1)

    hvec = np.zeros(D2, np.float64)
    for h in range(HEADS):
        sl = slice(h * 2 * HD, (h + 1) * 2 * HD)
        hvec[sl] = Wv[sl, :].astype(np.float64) @ (U[h] / S[h]) + bv[sl]
    out = hvec @ Wo.T.astype(np.float64) + bo
    return out.reshape(DIM, 2).astype(np.float32)
